# revision 5
# baseline (speedup 1.0000x reference)
"""FP4Linear forward for Trainium2, 8-way tensor-parallel.

y = x @ w_t  with x:[8192,4096] f32 and w_t:[4096,16384] f32 (w_t is the
exactly-consistent dequantized transposed weight supplied by the problem, so
no on-chip dequantization is needed).

Sharding (column-parallel per the hint): w_t is split along out_features into
8 shards of 2048; every core holds a replica of x and computes its own
y[:, c*2048:(c+1)*2048]; the host concatenates the slices.

Per-core kernel (bf16 matmuls, fp32 PSUM accumulation; ~2.3e-3 rel err):
  - Both operands pre-cast to bf16 on the host: halves DMA traffic vs fp32
    and lets the whole w shard (16.8 MB, 128 KiB/partition) stay resident in
    SBUF, so x is streamed exactly once.
  - ko-outer inner loop: each stationary x-slice is loaded once and reused by
    4 matmuls (one per 512-wide n-tile, accumulating in 4 PSUM banks).
    A/B probes measured ~30 ns/MM of exposed LDWEIGHTS cost with
    stationary-per-matmul ordering; this ordering removes it.
  - x m-tile loads ride the sync ring; w chunk loads and y stores ride the
    scalar ring so the x stream is never queued behind them. w is chunked
    along ko so the first m-tile can start after the first chunk lands.
"""

import ml_dtypes
import numpy as np

import concourse.mybir as mybir
import concourse.tile as tile
from concourse import bacc
from concourse.bass_utils import run_bass_kernel_spmd

P = 128
M_FULL, K_FULL, N_FULL = 8192, 4096, 16384
N_CORES = 8
N_PER = N_FULL // N_CORES  # 2048
KO = K_FULL // P  # 32
MT = M_FULL // P  # 64
FD = 512  # matmul moving free dim == one PSUM bank of fp32
NT = N_PER // FD  # 4
KC = 8  # ko per w-load chunk

_CACHE = {}


def build_nc(repeat=1):
    nc = bacc.Bacc("TRN2", target_bir_lowering=False, debug=False)
    dt = mybir.dt.bfloat16
    xd = nc.dram_tensor("x4", [MT, P, KO, P], dt, kind="ExternalInput")
    wd = nc.dram_tensor("w3", [P, KO, NT, FD], dt, kind="ExternalInput")
    yd = nc.dram_tensor("y3", [MT, P, N_PER], mybir.dt.float32,
                        kind="ExternalOutput")
    with tile.TileContext(nc) as tc:
        with (
            tc.tile_pool(name="wpool", bufs=1) as wpool,
            tc.tile_pool(name="xpool", bufs=3) as xpool,
            tc.tile_pool(name="opool", bufs=3) as opool,
            tc.tile_pool(name="psum", bufs=8, space="PSUM") as psum,
        ):
            # w is loop-invariant: load once, chunked along ko so the first
            # m-tile's matmuls can start as soon as chunk 0 lands.
            wt = wpool.tile([P, KO, NT, FD], dt, tag="wt")
            for c in range(KO // KC):
                nc.scalar.dma_start(
                    wt[:, c * KC : (c + 1) * KC],
                    wd[:, c * KC : (c + 1) * KC],
                )
            for _rep in range(repeat):
                for mt in range(MT):
                    xt = xpool.tile([P, KO, P], dt, tag="xt")
                    nc.sync.dma_start(xt[:], xd[mt])
                    ot = opool.tile([P, N_PER], mybir.dt.float32, tag="ot")
                    pss = []
                    for _nt in range(NT):
                        ps = psum.tile([P, FD], mybir.dt.float32, tag="ps")
                        pss.append(ps)
                    for ko in range(KO):
                        for nt in range(NT):
                            nc.tensor.matmul(
                                pss[nt][:],
                                xt[:, ko, :],
                                wt[:, ko, nt, :],
                                start=(ko == 0),
                                stop=(ko == KO - 1),
                            )
                    for nt in range(NT):
                        nc.vector.tensor_copy(
                            ot[:, nt * FD : (nt + 1) * FD], pss[nt][:]
                        )
                    nc.scalar.dma_start(yd[mt], ot[:])
    nc.compile()
    return nc


def prep_x(x):
    # [M, K] -> [MT, P(k), KO, P(m)]; elem [mt, p, ko, m] = x[mt*128+m, ko*128+p]
    a = np.ascontiguousarray(x, dtype=np.float32)
    return np.ascontiguousarray(
        a.reshape(MT, P, KO, P).transpose(0, 3, 2, 1).astype(ml_dtypes.bfloat16)
    )


def prep_w(w_slice):
    # [K, N_PER] -> [P(k), KO, NT, FD]; [p,ko,nt,f] = w[ko*128+p, nt*512+f]
    a = np.ascontiguousarray(w_slice, dtype=np.float32)
    return np.ascontiguousarray(
        a.reshape(KO, P, NT, FD).transpose(1, 0, 2, 3).astype(ml_dtypes.bfloat16)
    )


def kernel(x, w_q, w_os, w_is, w_t):
    if "nc" not in _CACHE:
        _CACHE["nc"] = build_nc(1)
    nc = _CACHE["nc"]

    xprep = prep_x(x)
    in_maps = [
        {"x4": xprep, "w3": prep_w(w_t[:, c * N_PER : (c + 1) * N_PER])}
        for c in range(N_CORES)
    ]
    res = run_bass_kernel_spmd(nc, in_maps, core_ids=list(range(N_CORES)))

    y = np.empty((M_FULL, N_FULL), dtype=np.float32)
    for c in range(N_CORES):
        y[:, c * N_PER : (c + 1) * N_PER] = (
            res.results[c]["y3"].reshape(M_FULL, N_PER)
        )
    return y


# revision 6
# speedup vs baseline: 1.1393x; 1.1393x over previous
"""FP4Linear forward for Trainium2, 8-way tensor-parallel.

y = x @ w_t  with x:[8192,4096] f32 and w_t:[4096,16384] f32 (w_t is the
exactly-consistent dequantized transposed weight supplied by the problem, so
no on-chip dequantization is needed).

Sharding (column-parallel per the hint): w_t is split along out_features into
8 shards of 2048; every core holds a replica of x and computes its own
y[:, c*2048:(c+1)*2048]; the host concatenates the slices.

Per-core kernel (bf16 matmuls, fp32 PSUM accumulation; ~2.3e-3 rel err):
  - Both operands pre-cast to bf16 on the host: halves DMA traffic vs fp32
    and lets the whole w shard (16.8 MB, 128 KiB/partition) stay resident in
    SBUF, so x is streamed exactly once (67 MB instead of 2x134 MB fp32).
  - x is pre-laid-out on host as [64, 128(k), 32(ko), 128(m)] so each m-tile
    load is one fully contiguous 1 MiB DMA on the sync ring; w chunks load on
    the scalar ring so they don't queue ahead of the first x tiles.
  - Inner loop: for each (m-tile, 512-wide n-tile): 32 accumulating matmuls
    over the contraction dim, PSUM -> SBUF copy on the vector engine, one
    2048-wide store DMA per m-tile on the scalar ring.
  - Measured ~1.73-1.84 ms on HW, ~99% of the 512-cycle-per-matmul roofline
    at 2.4 GHz (1.745 ms). A ko-outer variant that amortizes LDWEIGHTS 4x
    measured ~0.3 ms slower on HW despite a faster pure-PE probe; kept
    nt-outer.
"""

import ml_dtypes
import numpy as np

import concourse.mybir as mybir
import concourse.tile as tile
from concourse import bacc
from concourse.bass_utils import run_bass_kernel_spmd

P = 128
M_FULL, K_FULL, N_FULL = 8192, 4096, 16384
N_CORES = 8
N_PER = N_FULL // N_CORES  # 2048
KO = K_FULL // P  # 32
MT = M_FULL // P  # 64
FD = 512  # matmul moving free dim == one PSUM bank of fp32
NT = N_PER // FD  # 4

_CACHE = {}


def build_nc(repeat=1):
    nc = bacc.Bacc("TRN2", target_bir_lowering=False, debug=False)
    dt = mybir.dt.bfloat16
    xd = nc.dram_tensor("x4", [MT, P, KO, P], dt, kind="ExternalInput")
    wd = nc.dram_tensor("w3", [NT, P, KO, FD], dt, kind="ExternalInput")
    yd = nc.dram_tensor("y3", [MT, P, N_PER], mybir.dt.float32,
                        kind="ExternalOutput")
    with tile.TileContext(nc) as tc:
        with (
            tc.tile_pool(name="wpool", bufs=1) as wpool,
            tc.tile_pool(name="xpool", bufs=3) as xpool,
            tc.tile_pool(name="opool", bufs=3) as opool,
            tc.tile_pool(name="psum", bufs=8, space="PSUM") as psum,
        ):
            # w is loop-invariant: load it once, before the repeat loop.
            # nt=0 chunk split by ko so the first matmuls start early;
            # remaining chunks load whole. All on the scalar ring so the
            # x-tile loads on the sync ring aren't queued behind them.
            wt = wpool.tile([P, NT, KO, FD], dt, tag="wt")
            for c in range(4):
                nc.scalar.dma_start(
                    wt[:, 0, c * (KO // 4) : (c + 1) * (KO // 4), :],
                    wd[0, :, c * (KO // 4) : (c + 1) * (KO // 4), :],
                )
            for nt in range(1, NT):
                nc.scalar.dma_start(wt[:, nt], wd[nt])
            for _rep in range(repeat):
                for mt in range(MT):
                    xt = xpool.tile([P, KO, P], dt, tag="xt")
                    nc.sync.dma_start(xt[:], xd[mt])
                    ot = opool.tile([P, N_PER], mybir.dt.float32, tag="ot")
                    for nt in range(NT):
                        ps = psum.tile([P, FD], mybir.dt.float32, tag="ps")
                        for ko in range(KO):
                            nc.tensor.matmul(
                                ps[:],
                                xt[:, ko, :],
                                wt[:, nt, ko, :],
                                start=(ko == 0),
                                stop=(ko == KO - 1),
                            )
                        nc.vector.tensor_copy(
                            ot[:, nt * FD : (nt + 1) * FD], ps[:]
                        )
                    nc.scalar.dma_start(yd[mt], ot[:])
    nc.compile()
    return nc


def prep_x(x):
    # [M, K] -> [MT, P(k), KO, P(m)]; elem [mt, p, ko, m] = x[mt*128+m, ko*128+p]
    a = np.ascontiguousarray(x, dtype=np.float32)
    return np.ascontiguousarray(
        a.reshape(MT, P, KO, P).transpose(0, 3, 2, 1).astype(ml_dtypes.bfloat16)
    )


def prep_w(w_slice):
    # [K, N_PER] -> [NT, P(k), KO, FD]; [nt,p,ko,f] = w[ko*128+p, nt*512+f]
    a = np.ascontiguousarray(w_slice, dtype=np.float32)
    return np.ascontiguousarray(
        a.reshape(KO, P, NT, FD).transpose(2, 1, 0, 3).astype(ml_dtypes.bfloat16)
    )


def kernel(x, w_q, w_os, w_is, w_t):
    if "nc" not in _CACHE:
        _CACHE["nc"] = build_nc(1)
    nc = _CACHE["nc"]

    xprep = prep_x(x)
    in_maps = [
        {"x4": xprep, "w3": prep_w(w_t[:, c * N_PER : (c + 1) * N_PER])}
        for c in range(N_CORES)
    ]
    res = run_bass_kernel_spmd(nc, in_maps, core_ids=list(range(N_CORES)))

    y = np.empty((M_FULL, N_FULL), dtype=np.float32)
    for c in range(N_CORES):
        y[:, c * N_PER : (c + 1) * N_PER] = (
            res.results[c]["y3"].reshape(M_FULL, N_PER)
        )
    return y
